# revision 13
# baseline (speedup 1.0000x reference)
"""Masked dot-product attention (B=16, LQ=LK=2048, D=64) on 8 TRN2 NeuronCores.

Strategy
--------
out[b] = softmax(mask(Q K^T / 8)) V, keys >= valid_len[b] masked.

Work decomposition (as the previous ACT-bound version): each (batch,
q-half-of-1024) job costs ceil(valid_len/128) k-tiles; jobs are split into
segments of <= 8 k-tiles, sorted and dealt 8-at-a-time into slot ranks so
all cores run one instruction stream while tracking the sparsity.

Per (slot, k-tile) on device, all inputs fp16:
  MM1   S^T[kk, q] = (K^T chunk).T @ Q^T      (d=64 contraction, fp16, f32 acc)
  EXP   P[kk, q]   = exp(S^T / 8) as fp16     (engine varies, see below)
  MM2   acc[q, dd|1] += P_chunk.T @ [V|1]     (k contraction, PSUM accum)

MM2 uses P 128-column chunks as the *stationary* tensor and [V|ones] [128,65]
as the *moving* tensor, so it costs 4x65 moving rows per 512 q instead of
512: with fp16 (1 PE cycle/row at any size) MM2 drops from 427ns to ~220ns
per k-tile.  The accumulator acc[q, d] comes out q-major, so the host-side
combine needs no transpose.

EXP runs on two engines to beat the ScalarE throughput wall:
 - ACT: true exp activation (exact to fp16).
 - DVE: Schraudolph-style bit-trick exp in ONE fused tensor_scalar:
     z = S^T * (log2(e)/8 * 1024) + (2^23 + 15*1024 + sigma)
   The f32 add at 2^23 rounds to integer; the low int16 of each f32 word is
   then exactly the bit pattern of fp16(2^((j-15360)/1024)) ~= exp(s/8) with
   <= ~3% relative error. MM2 reads P through a stride-2 int16 view bitcast
   to fp16 -- no extra instruction.
  Trick tiles are only scheduled on ranks whose source batches all have
  >= ELIG_NMIN k-tiles (small-valid-len batches average the trick noise
  less, and absmax error is dominated by them), and engine assignment
  greedily balances ACT/DVE busy time in execution order.

Masking: V rows and the ones-column are zeroed host-side for keys >=
valid_len or outside the segment, so masked keys contribute exactly 0 to
numerator and denominator.  Each slot DMAs its raw [2x128, 4x65]
accumulator out; the host sums each job's segments in f64 and divides.
"""

import math
from contextlib import ExitStack

import numpy as np

import concourse.bacc as bacc
import concourse.mybir as mybir
import concourse.tile as tile
import concourse.bass_utils as bass_utils

B, LQ, LK, D = 16, 2048, 2048, 64
N_CORES = 8
KT = 128          # keys per k-tile
QS = 1024         # queries per slot (q-half)
SEG = 8           # max k-tiles per segment
SCALE = 1.0 / math.sqrt(D)

F32 = mybir.dt.float32
F16 = mybir.dt.float16
I16 = mybir.dt.int16
MM_DT = F16

# Schraudolph fp16 trick constants (input = RAW scores, scale folded in).
# D tier (1 DVE op):  P-bits j = round_i16(s*C_TRICK + 15360 + SIG_D).
# H tier (2-phase average, 4 ops): j0 = round_i16(s*C_TRICK + 15360-1024+SIG_H)
#   j2 = j0-512; P = f16(j0) + f16(j2)*sqrt(2) -- averages the Schraudolph
#   interp error at phases t and t-1/2, cutting its centered rms from ~2.1%
#   to ~0.47%.
# Each tier's P carries a constant factor (2^(SIG/1024) and the mean interp
# bias); the host folds 1/factor into that k-tile's V slab (V_SCALE_*), so
# tiers mix exactly.
C_TRICK = SCALE * math.log2(math.e) * 1024.0          # 184.665
SIG_D = -44.0
SIG_H = -44.0
MAGIC_D = 15360.0 + SIG_D
MAGIC_H = 15360.0 - 1024.0 + SIG_H
SQRT2 = float(np.float32(math.sqrt(2.0)))
MEAN_D = 1.01022   # E[P/e^x] / 2^(SIG/1024), x~N(0,1), measured numerically
MEAN_H = 1.04058
V_SCALE_D = 1.0 / (2.0 ** (SIG_D / 1024.0) * MEAN_D)
V_SCALE_H = 1.0 / (2.0 ** (SIG_H / 1024.0) * MEAN_H)
# Trick-exp noise multiplies softmax weights; end-to-end absmax-rel error
# fits err_b ~= coef * sqrt(share_b * 964 / vl_b) with coef 2.1e-2 for the
# D tier and 4.8e-3 for the H tier.  Per-batch share caps hold each tier's
# contribution to T1=8e-3 / T2=7.5e-3 (quadrature sum ~1.1e-2 < 2e-2 gate).
CAP_D = (8.0e-3 / 2.1e-2) ** 2 / 964.0
CAP_H = (7.5e-3 / 4.8e-3) ** 2 / 964.0

# engine cost model (ns) for the greedy balancer
ACT_EXP_NS = 1038.0
DVE_EXP_NS = 1192.0     # D tier: 1 tensor_scalar, f32 psum -> i16
DVE_H_NS = 1846.0       # H tier DVE ops: op1 1192 + op2/op3 ~327 each
DVE_TT_NS = 593.0       # H op4 on DVE (f16 tensor_tensor, 2x)
POOL_TT_NS = 1553.0     # H op4 on Pool
ACT_CPY_NS = 402.0
DVE_CPY_NS = 396.0


def pair_layout(rank_lens, j):
    """fp16-element column offsets of the sections inside pair tensor j.

    Sections: qT (both slots stacked on partition halves) | kT (same) |
    vp[slot 2j] | vp[slot 2j+1] ([V|1] per k-tile, 65 cols each, zeroed for
    masked / out-of-segment keys).
    """
    na, nb = rank_lens[2 * j], rank_lens[2 * j + 1]
    qo = 0
    ko = qo + QS
    vo = [ko + na * KT, ko + na * KT + na * (D + 1)]
    width = vo[1] + nb * (D + 1)
    return qo, ko, vo, width


def slot_order(rank_lens):
    """Execution order: shortest slots first (less DMA before compute)."""
    return sorted(range(len(rank_lens)), key=lambda s: rank_lens[s])


def assign_engines(rank_lens, quotas):
    """Three-tier exp schedule under per-slot accuracy quotas.

    quotas[s] = (qd, qh): max D-tier / H-tier tiles in slot s.  Greedy
    minimizes the max engine load; 'H' tiles split their op4 between Pool
    and DVE.  Returns (exp_eng {(s,kt): 'A'|'D'|'H'|'HP'}, epi_eng
    {(s,qq): 'A'|'D'}).  'H' = op4 on DVE, 'HP' = op4 on Pool.
    """
    exp_eng = {}
    epi_eng = {}
    load = {"A": 0.0, "D": 0.0, "P": 0.0}
    for s in slot_order(rank_lens):
        ns = rank_lens[s]
        qd, qh = quotas[s]
        for kt in range(ns):
            cands = [("A", load["A"] + ACT_EXP_NS, ("A", ACT_EXP_NS, 0.0))]
            nd = sum(
                1 for k in range(kt) if exp_eng[(s, k)] == "D"
            )
            nh = sum(
                1 for k in range(kt) if exp_eng[(s, k)] in ("H", "HP")
            )
            if nd < qd:
                cands.append(
                    ("D", load["D"] + DVE_EXP_NS, ("D", DVE_EXP_NS, 0.0))
                )
            if nh < qh:
                cands.append(
                    (
                        "HP",
                        max(load["D"] + DVE_H_NS, load["P"] + POOL_TT_NS),
                        ("HP", DVE_H_NS, POOL_TT_NS),
                    )
                )
                cands.append(
                    ("H", load["D"] + DVE_H_NS + DVE_TT_NS, ("H", DVE_H_NS + DVE_TT_NS, 0.0))
                )
            tier, _, (t, dcost, pcost) = min(cands, key=lambda c: c[1])
            exp_eng[(s, kt)] = t
            if t == "A":
                load["A"] += ACT_EXP_NS
            else:
                load["D"] += dcost
                load["P"] += pcost
        for qq in range(2):
            if load["A"] + ACT_CPY_NS <= load["D"] + DVE_CPY_NS:
                epi_eng[(s, qq)] = "A"
                load["A"] += ACT_CPY_NS
            else:
                epi_eng[(s, qq)] = "D"
                load["D"] += DVE_CPY_NS
    return exp_eng, epi_eng


def exp_model(tier, sraw):
    """Numpy model of the device exp for one tile: sraw [128, nq] raw
    scores (f32), returns P as float64 (matching fp16/bit-trick device
    numerics).  Used by the packer (V prescale is exact regardless) and by
    test emulation."""
    if tier == "A":
        return (
            np.exp(sraw.astype(np.float32) * np.float32(SCALE))
            .astype(np.float16)
            .astype(np.float64)
        )
    prod = (sraw.astype(np.float32) * np.float32(C_TRICK)).astype(np.float32)
    if tier == "D":
        j = np.round(prod.astype(np.float64) + MAGIC_D).astype(np.int64)
        return (
            np.ascontiguousarray(j.astype(np.int16)).view(np.float16).astype(np.float64)
        )
    j0 = np.round(prod.astype(np.float64) + MAGIC_H).astype(np.int64)
    t1 = np.ascontiguousarray(j0.astype(np.int16)).view(np.float16)
    t2 = np.ascontiguousarray((j0 - 512).astype(np.int16)).view(np.float16)
    return (t1 + t2 * np.float16(SQRT2)).astype(np.float64)


def build_bass(plan, repeat=1, cfg=None):
    """Build the per-core Bass program.

    plan = (rank_lens, quotas): compiled slot lengths + per-slot DVE
    trick-exp tile quotas.  repeat>1 re-runs everything (timing only).
    """
    rank_lens, quotas = plan
    cf = {"sp": 2, "ap": 4, "pp": 10, "zp": 10, "ep": 3}
    if cfg:
        cf.update(cfg)
    slots = len(rank_lens)
    pairs = slots // 2
    nc = bacc.Bacc("TRN2", target_bir_lowering=False, debug=False)

    widths = [pair_layout(rank_lens, j)[3] for j in range(pairs)]
    qk_w = [pair_layout(rank_lens, j)[2][0] for j in range(pairs)]
    pk = [
        nc.dram_tensor(f"pk{j}", [128, widths[j]], MM_DT, kind="ExternalInput").ap()
        for j in range(pairs)
    ]
    out = nc.dram_tensor("out", [slots * 2 * 128, 4 * (D + 1)], F32, kind="ExternalOutput").ap()

    Exp = mybir.ActivationFunctionType.Exp
    exp_eng, epi_eng = assign_engines(rank_lens, quotas)

    with tile.TileContext(nc) as tc, ExitStack() as ctx:
        inp = ctx.enter_context(tc.tile_pool(name="inp", bufs=1))
        ppool = ctx.enter_context(tc.tile_pool(name="pp", bufs=cf["pp"]))
        zpool = ctx.enter_context(tc.tile_pool(name="zp", bufs=cf["zp"]))
        epool = ctx.enter_context(tc.tile_pool(name="ep", bufs=cf["ep"]))
        spool = ctx.enter_context(tc.tile_pool(name="sp", bufs=cf["sp"], space="PSUM"))
        apool = ctx.enter_context(tc.tile_pool(name="ap", bufs=cf["ap"], space="PSUM"))

        order = cf.get("order") or slot_order(rank_lens)
        pair_order = sorted(range(pairs), key=lambda j: rank_lens[2 * j])

        qk_t = [None] * pairs
        km_t = [None] * pairs   # middle k-columns (3-way-split first pair)
        kx_t = [None] * pairs   # overflow k-columns beyond the shorter slot
        kx_at = [None] * pairs  # k-tile index where the overflow tile starts
        v_t = [None] * pairs
        j0 = pair_order[0]
        dma_seq = cf.get("dma_seq") or [
            (kind, j) for j in pair_order for kind in ("qk", "v")
        ]
        for kind, j in dma_seq:
            if kind == "qk":
                na, nb = rank_lens[2 * j], rank_lens[2 * j + 1]
                if j in pair_order[:2] and na > nb and nb > 1:
                    # 3-way split for the startup-critical first pairs: the
                    # very first matmul waits only for q + one k-tile.
                    w1 = QS + KT
                    qk_t[j] = inp.tile([128, w1], MM_DT, name=f"qk{j}")
                    nc.sync.dma_start(qk_t[j][:], pk[j][:, :w1])
                    km_t[j] = inp.tile([128, (nb - 1) * KT], MM_DT, name=f"km{j}")
                    nc.sync.dma_start(km_t[j][:], pk[j][:, w1 : QS + nb * KT])
                    kx_t[j] = inp.tile([128, (na - nb) * KT], MM_DT, name=f"kx{j}")
                    kx_at[j] = nb
                    nc.sync.dma_start(kx_t[j][:], pk[j][:, QS + nb * KT : qk_w[j]])
                elif na > nb and (j == j0 or cf.get("split_all", True)):
                    wa = QS + nb * KT
                    qk_t[j] = inp.tile([128, wa], MM_DT, name=f"qk{j}")
                    nc.sync.dma_start(qk_t[j][:], pk[j][:, :wa])
                    kx_t[j] = inp.tile([128, (na - nb) * KT], MM_DT, name=f"kx{j}")
                    kx_at[j] = nb
                    nc.sync.dma_start(kx_t[j][:], pk[j][:, wa : qk_w[j]])
                else:
                    qk_t[j] = inp.tile([128, qk_w[j]], MM_DT, name=f"qk{j}")
                    nc.sync.dma_start(qk_t[j][:], pk[j][:, : qk_w[j]])
            else:
                v_t[j] = inp.tile([128, widths[j] - qk_w[j]], MM_DT, name=f"v{j}")
                nc.sync.dma_start(v_t[j][:], pk[j][:, qk_w[j] :])

        for s in [s for _ in range(repeat) for s in order]:
            ns = rank_lens[s]
            j = s // 2          # pair index (shared input tile)
            pb = (s % 2) * 64   # partition base for q/k sections
            qo, ko, vo, _ = pair_layout(rank_lens, j)
            voff = vo[s % 2] - qk_w[j]
            pt = qk_t[j]
            accs = [
                apool.tile([128, 4 * (D + 1)], F32, name=f"acc{s}_{qq}", tag="acc")
                for qq in range(2)
            ]

            p_aps = []
            for kt in range(ns):
                s_ps = spool.tile([128, QS], F32, name="s_ps")
                if kx_at[j] is not None and kt >= kx_at[j]:
                    kk = kt - kx_at[j]
                    lhsT = kx_t[j][pb : pb + 64, kk * KT : (kk + 1) * KT]
                elif km_t[j] is not None and kt >= 1:
                    lhsT = km_t[j][pb : pb + 64, (kt - 1) * KT : kt * KT]
                else:
                    lhsT = pt[pb : pb + 64, ko + kt * KT : ko + (kt + 1) * KT]
                for qq in range(2):
                    nc.tensor.matmul(
                        s_ps[:, qq * 512 : (qq + 1) * 512],
                        lhsT,
                        pt[pb : pb + 64, qo + qq * 512 : qo + (qq + 1) * 512],
                        start=True,
                        stop=True,
                    )
                if exp_eng[(s, kt)] == "A":
                    p_t = ppool.tile([128, QS], F16, name="p_t")
                    nc.scalar.activation(p_t[:], s_ps[:], Exp, scale=SCALE)
                    p_aps.append(p_t[:])
                else:
                    z_t = zpool.tile([128, QS], F32, name="z_t")
                    nc.vector.tensor_scalar(
                        z_t[:], s_ps[:], C_TRICK, MAGIC,
                        mybir.AluOpType.mult, mybir.AluOpType.add,
                    )
                    p_aps.append(z_t[:].bitcast(I16)[:, 0 : 2 * QS : 2].bitcast(F16))

            # MM2 sweep: one psum accumulation group per (qq, chunk) at a
            # time (CoreSim/HW allow one open group per 2KB zero region);
            # each group contracts over all k-tiles before the next starts.
            for qq in range(2):
                for c in range(4):
                    g = qq * 4 + c
                    for kt in range(ns):
                        w = v_t[j][
                            :, voff + kt * (D + 1) : voff + (kt + 1) * (D + 1)
                        ]
                        nc.tensor.matmul(
                            accs[qq][:, c * (D + 1) : (c + 1) * (D + 1)],
                            p_aps[kt][:, g * 128 : (g + 1) * 128],
                            w,
                            start=(kt == 0),
                            stop=(kt == ns - 1),
                        )

            # Epilogue: raw partial accumulators straight to DRAM
            # (host does the segment-sum + divide).
            for qq in range(2):
                acc_sb = epool.tile([128, 4 * (D + 1)], F32, name="acc_sb")
                if epi_eng[(s, qq)] == "A":
                    nc.scalar.copy(acc_sb[:], accs[qq][:])
                else:
                    nc.vector.tensor_copy(acc_sb[:], accs[qq][:])
                r0 = (s * 2 + qq) * 128
                nc.sync.dma_start(out[r0 : r0 + 128, :], acc_sb[:])

    nc.compile()
    return nc


def plan_and_pack(queries, keys, values, valid_lens):
    """Split jobs into k-segments, deal into rank slots, gather fp16 inputs."""
    q = np.ascontiguousarray(np.asarray(queries, dtype=np.float32))
    k = np.asarray(keys, dtype=np.float32)
    v = np.asarray(values, dtype=np.float32)
    vl = np.asarray(valid_lens, dtype=np.int64)

    nkt = np.maximum(1, -(-vl // KT))  # ceil

    def make_segs(seg_max):
        segs = []  # (len, b, qh, k0)
        for b in range(B):
            n = int(nkt[b])
            m = -(-n // seg_max)
            base, rem = divmod(n, m)
            sizes = [base + 1] * rem + [base] * (m - rem)
            for qh in range(LQ // QS):
                k0 = 0
                for sz in sizes:
                    segs.append((sz, b, qh, k0))
                    k0 += sz
        segs.sort(key=lambda t: (-t[0], t[1], t[2], t[3]))
        return segs

    def cost(segs):
        ls = sorted((s[0] for s in segs), reverse=True)
        while len(ls) % N_CORES:
            ls.append(0)
        slots = len(ls) // N_CORES
        if slots % 2:
            slots += 1
            ls += [0] * N_CORES
        rsum = sum(max(ls[N_CORES * r], 1) for r in range(slots))
        return rsum * 0.68 + slots * 0.55  # us: PE-paced unit + slot overhead

    seg_best = min(range(5, SEG + 1), key=lambda m: cost(make_segs(m)))
    segs = make_segs(seg_best)
    while len(segs) % N_CORES:
        segs.append(None)
    slots = len(segs) // N_CORES
    if slots % 2:  # pair structure needs an even slot count
        segs.extend([None] * N_CORES)
        slots += 1
    rank_lens = []
    quotas = []
    for r in range(slots):
        rsegs = segs[N_CORES * r : N_CORES * (r + 1)]
        first = rsegs[0]
        ns = first[0] if first is not None else 1
        rank_lens.append(ns)
        min_vl = min(
            (int(vl[sg[1]]) for sg in rsegs if sg is not None), default=LK
        )
        quotas.append(int(ns * CAP_COEF * min_vl))
    pairs = slots // 2

    kT = np.swapaxes(k, 1, 2)  # [B, D, LK] view
    parts = np.arange(KT)

    in_maps = []
    slot_map = []  # per core: [(b, qh, k0) or None, ...] per slot
    for c in range(N_CORES):
        core_map = {}
        smap = []
        for j in range(pairs):
            qo, ko, vo, width = pair_layout(rank_lens, j)
            pkj = np.zeros((128, width), dtype=np.float16)
            for i, s in enumerate((2 * j, 2 * j + 1)):
                nr = rank_lens[s]  # compiled (padded) slot length
                seg = segs[N_CORES * s + c]
                if seg is None:
                    smap.append(None)
                    continue
                sz, b, qh, k0 = seg
                pb = i * 64
                smap.append((b, qh, k0))
                pkj[pb : pb + 64, qo : qo + QS] = q[b, qh * QS : (qh + 1) * QS, :].T
                kw = min(nr * KT, LK - k0 * KT)
                pkj[pb : pb + 64, ko : ko + kw] = kT[b, :, k0 * KT : k0 * KT + kw]
                vslab = pkj[:, vo[i] : vo[i] + nr * (D + 1)].reshape(128, nr, D + 1)
                nv = kw // KT
                vslab[:, :nv, :D] = (
                    v[b, k0 * KT : k0 * KT + nv * KT, :]
                    .reshape(nv, KT, D)
                    .transpose(1, 0, 2)
                )
                vslab[:, :, D] = 1.0
                # zero contributions of masked keys and keys outside the
                # segment's own range [k0, k0+sz)
                kid = (k0 + np.arange(nr))[None, :] * KT + parts[:, None]
                dead = (kid >= vl[b]) | (kid >= (k0 + sz) * KT)
                vslab[dead] = 0.0
            core_map[f"pk{j}"] = pkj
        in_maps.append(core_map)
        slot_map.append(smap)
    return (rank_lens, quotas), in_maps, slot_map


def scatter_out(results, slot_map):
    acc = {}  # (b, qh) -> [QS, 65] float64 partial sums
    for c in range(N_CORES):
        oc = results[c]["out"]
        for s, seg in enumerate(slot_map[c]):
            if seg is None:
                continue
            b, qh, _ = seg
            blk = oc[s * 256 : (s + 1) * 256, :].astype(np.float64)
            # rows: [2 qq x 128 p], cols: [4 chunks x 65] -> [1024 q, 65]
            jb = (
                blk.reshape(2, 128, 4, D + 1)
                .transpose(0, 2, 1, 3)
                .reshape(QS, D + 1)
            )
            key = (b, qh)
            if key in acc:
                acc[key] += jb
            else:
                acc[key] = jb
    out = np.empty((B, LQ, D), dtype=np.float32)
    for (b, qh), a in acc.items():
        out[b, qh * QS : (qh + 1) * QS, :] = a[:, :D] / a[:, D : D + 1]
    return out


def kernel(queries, keys, values, valid_lens, _run=None):
    plan, in_maps, slot_map = plan_and_pack(queries, keys, values, valid_lens)
    nc = build_bass(plan)
    if _run is not None:  # test hook (e.g. CoreSim)
        results = _run(nc, in_maps)
    else:
        import time as _time

        last = None
        for attempt in range(4):  # axon devices flake transiently under load
            try:
                results = bass_utils.run_bass_kernel_spmd(
                    nc, in_maps, core_ids=list(range(N_CORES))
                ).results
                break
            except Exception as e:  # noqa: BLE001
                last = e
                _time.sleep(45.0 * (attempt + 1))
        else:
            raise last
    return scatter_out(results, slot_map)


# revision 44
# speedup vs baseline: 1.1939x; 1.1939x over previous
"""Masked dot-product attention (B=16, LQ=LK=2048, D=64) on 8 TRN2 NeuronCores.

Strategy
--------
out[b] = softmax(mask(Q K^T / 8)) V, keys >= valid_len[b] masked.

Work decomposition: each (batch, q-half-of-1024) job costs
ceil(valid_len/128) k-tiles; jobs split into segments of <= 8 k-tiles,
sorted and dealt 8-at-a-time into slot ranks so all cores run one
instruction stream while total work tracks the sparsity.

Per (slot, k-tile) on device, all inputs fp16:
  MM1   S^T[kk, q] = (K^T chunk).T @ Q^T      (d=64 contraction, f32 acc)
  EXP   P[kk, q]   = exp(S^T / 8) as fp16     (three engine tiers, below)
  MM2   acc[q, dd|1] += P_chunk.T @ [V|1]     (k contraction, PSUM accum)

MM2 uses P 128-column chunks as the *stationary* tensor and [V|ones]
[128,65] as the *moving* tensor: 8x65 moving rows per k-tile instead of
2x512 (matmul time ~ moving rows; fp16 runs 1 PE cycle/row at any size),
cutting MM2 from 854ns to ~220ns/k-tile.  acc comes out q-major.  Since
only one PSUM accumulation group may be open per 2KB bank, each (qq,
chunk) group contracts over all its k-tiles and closes before the next
opens; the whole 8-group sweep is deferred one slot and spliced between
the NEXT slot's MM1/exp stages (in-order PE overlaps it with the exp
engines), with contraction in tier-readiness order so a lagging exp never
head-of-line-blocks the PE.

EXP runs on three engine tiers to beat the ScalarE throughput wall
(ScalarE exp costs 1 elem/cycle/lane; ~36us for this shape on one engine):
 - A (ACT): true exp activation, exact to fp16.
 - D (DVE, 1 op): Schraudolph bit-trick: j = round_i16(s*C + 15360+sig);
   j IS the bit pattern of fp16(~exp(s/8)) with ~1.8% rms error; MM2 reads
   the int16 tile bitcast to fp16.
 - H (DVE + Pool, 3 ops): j0 as above (-1024 bias), j2 = j0-512, then
   P = f16(j0)+f16(j2) (tensor_tensor, on Pool for some tiles): averages
   the interp error at phases half a period apart -> ~0.53% rms.
Each tier's constant bias factor is folded into that k-tile's V slab
host-side (exact), so tiers mix freely.  Per-batch tile-share caps
(prop. to valid_len) keep the end-to-end absmax error ~6e-3 (gate 2e-2);
a greedy balancer assigns tiers under these caps to equalize engine
loads, interleaving tiers within each slot (the s_ps PSUM ring forces
strict k-tile order, so same-engine runs would serialize).

Also: PE p-state warmup matmuls during the input-DMA window (the cost
model's clock ramp persists), fp16 inputs halve DMA, input pair tensors
stream in execution order with the startup-critical first transfers split
fine.  Masking: V rows and the ones-column are zeroed host-side for keys
>= valid_len or outside the segment.  Each slot DMAs its raw [2x128,
4x65] f32 accumulator out; the host sums each job's segments in f64 and
divides (exact).  TimelineSim: ~41.0us vs 47.1us for the previous
ACT-bound f32r kernel.
"""

import math
from contextlib import ExitStack

import numpy as np

import concourse.bacc as bacc
import concourse.mybir as mybir
import concourse.tile as tile
import concourse.bass_utils as bass_utils

B, LQ, LK, D = 16, 2048, 2048, 64
N_CORES = 8
KT = 128          # keys per k-tile
QS = 1024         # queries per slot (q-half)
SEG = 8           # max k-tiles per segment
SCALE = 1.0 / math.sqrt(D)

F32 = mybir.dt.float32
F16 = mybir.dt.float16
I16 = mybir.dt.int16
MM_DT = F16

# Schraudolph fp16 trick constants (input = RAW scores, scale folded in).
# D tier (1 DVE op):  P-bits j = round_i16(s*C_TRICK + 15360 + SIG_D).
# H tier (2-phase sum, 3 ops): j0 = round_i16(s*C_TRICK + 15360-1024+SIG_H);
#   j2 = j0-512; P = f16(j0) + f16(j2) -- a 0.59/0.41-weighted average of
#   the Schraudolph interp error at phases t and t-1/2, cutting the
#   centered rms from ~1.8% to ~0.53%.
# Each tier's P carries a constant total factor E[P/e^x] (measured
# numerically over x~N(0,1)); the host folds 1/factor into that k-tile's V
# slab (V_SCALE_*), so tiers mix exactly.
C_TRICK = SCALE * math.log2(math.e) * 1024.0          # 184.665
SIG_D = -44.0
SIG_H = -44.0
MAGIC_D = 15360.0 + SIG_D
MAGIC_H = 15360.0 - 1024.0 + SIG_H
V_SCALE_D = 1.0 / 1.0101518
V_SCALE_H = 1.0 / 0.8622136
# Trick-exp noise multiplies softmax weights; end-to-end absmax-rel error
# fits err_b ~= coef * sqrt(share_b * 964 / vl_b) with coef 2.1e-2 for the
# D tier (delta_rms 1.77%) and 6.35e-3 for the H tier (0.535%).  Per-batch
# share caps hold each tier's contribution to T1=8e-3 / T2=7.5e-3
# (quadrature sum ~1.1e-2 < 2e-2 gate).
CAP_D = (9.0e-3 / 2.1e-2) ** 2 / 964.0
CAP_H = (1.1e-2 / 6.35e-3) ** 2 / 964.0

# engine cost model (ns) for the greedy balancer (incl. seq overheads,
# calibrated against TimelineSim busy totals)
ACT_EXP_NS = 1100.0
DVE_EXP_NS = 1290.0     # D tier: 1 tensor_scalar, f32 psum -> i16
DVE_H_NS = 1620.0       # H tier DVE ops: op1 + op2
DVE_TT_NS = 650.0       # H op4 on DVE (f16 tensor_tensor, 2x)
POOL_TT_NS = 1650.0     # H op4 on Pool
ACT_CPY_NS = 440.0
DVE_CPY_NS = 450.0


def pair_layout(rank_lens, j):
    """fp16-element column offsets of the sections inside pair tensor j.

    Sections: qT (both slots stacked on partition halves) | kT (same) |
    vp[slot 2j] | vp[slot 2j+1] ([V|1] per k-tile, 65 cols each, zeroed for
    masked / out-of-segment keys).
    """
    na, nb = rank_lens[2 * j], rank_lens[2 * j + 1]
    qo = 0
    ko = qo + QS
    vo = [ko + na * KT, ko + na * KT + na * (D + 1)]
    width = vo[1] + nb * (D + 1)
    return qo, ko, vo, width


def slot_order(rank_lens):
    """Execution order: shortest slots first (less DMA before compute)."""
    return sorted(range(len(rank_lens)), key=lambda s: rank_lens[s])


def assign_engines(rank_lens, quotas):
    """Three-tier exp schedule under per-slot accuracy quotas.

    quotas[s] = (qd, qh): max D-tier / H-tier tiles in slot s.  Greedy
    minimizes the max engine load; 'H' tiles split their op4 between Pool
    and DVE.  Returns (exp_eng {(s,kt): 'A'|'D'|'H'|'HP'}, epi_eng
    {(s,qq): 'A'|'D'}).  'H' = op4 on DVE, 'HP' = op4 on Pool.
    """
    exp_eng = {}
    epi_eng = {}
    load = {"A": 0.0, "D": 0.0, "P": 0.0}
    order = slot_order(rank_lens)
    for s in order:
        ns = rank_lens[s]
        qd, qh = quotas[s]
        # pass 1: per-slot tier counts by global engine-load balance
        counts = {"A": 0, "D": 0, "H": 0, "HP": 0}
        for _ in range(ns):
            cands = [("A", load["A"] + ACT_EXP_NS, ACT_EXP_NS, 0.0)]
            if counts["D"] < qd:
                cands.append(("D", load["D"] + DVE_EXP_NS, DVE_EXP_NS, 0.0))
            if counts["H"] + counts["HP"] < qh:
                if load["P"] + POOL_TT_NS <= load["D"] + DVE_H_NS + 1000.0:
                    cands.append(
                        (
                            "HP",
                            max(load["D"] + DVE_H_NS, load["P"] + POOL_TT_NS),
                            DVE_H_NS,
                            POOL_TT_NS,
                        )
                    )
                cands.append(
                    ("H", load["D"] + DVE_H_NS + DVE_TT_NS, DVE_H_NS + DVE_TT_NS, 0.0)
                )
            t, _, dcost, pcost = min(cands, key=lambda c: c[1])
            counts[t] += 1
            if t == "A":
                load["A"] += ACT_EXP_NS
            else:
                load["D"] += dcost
                load["P"] += pcost
        # pass 2: interleave tiers evenly within the slot (strict kt-order
        # processing through the s_ps ring serializes same-engine runs)
        seq = []
        rem = dict(counts)
        placed = {t: 0 for t in rem}
        for kt in range(ns):
            t = max(
                (t for t in rem if rem[t] > 0),
                key=lambda t: rem[t] / (placed[t] + 1),
            )
            seq.append(t)
            rem[t] -= 1
            placed[t] += 1
        for kt, t in enumerate(seq):
            exp_eng[(s, kt)] = t
        for qq in range(2):
            if load["A"] + ACT_CPY_NS <= load["D"] + DVE_CPY_NS:
                epi_eng[(s, qq)] = "A"
                load["A"] += ACT_CPY_NS
            else:
                epi_eng[(s, qq)] = "D"
                load["D"] += DVE_CPY_NS
    return exp_eng, epi_eng


def exp_model(tier, sraw):
    """Numpy model of the device exp for one tile: sraw [128, nq] raw
    scores (f32), returns P as float64 (matching fp16/bit-trick device
    numerics).  Used by the packer (V prescale is exact regardless) and by
    test emulation."""
    if tier == "A":
        return (
            np.exp(sraw.astype(np.float32) * np.float32(SCALE))
            .astype(np.float16)
            .astype(np.float64)
        )
    prod = (sraw.astype(np.float32) * np.float32(C_TRICK)).astype(np.float32)
    if tier == "D":
        j = np.round(prod.astype(np.float64) + MAGIC_D).astype(np.int64)
        return (
            np.ascontiguousarray(j.astype(np.int16)).view(np.float16).astype(np.float64)
        )
    j0 = np.round(prod.astype(np.float64) + MAGIC_H).astype(np.int64)
    t1 = np.ascontiguousarray(j0.astype(np.int16)).view(np.float16)
    t2 = np.ascontiguousarray((j0 - 512).astype(np.int16)).view(np.float16)
    return (t1 + t2).astype(np.float64)


def build_bass(plan, repeat=1, cfg=None):
    """Build the per-core Bass program.

    plan = (rank_lens, quotas): compiled slot lengths + per-slot DVE
    trick-exp tile quotas.  repeat>1 re-runs everything (timing only).
    """
    rank_lens, quotas = plan
    cf = {"sp": 3, "ap": 2, "pp": 18, "zp": 18, "hp": 4, "ep": 3}
    if cfg:
        cf.update(cfg)
    slots = len(rank_lens)
    pairs = slots // 2
    nc = bacc.Bacc("TRN2", target_bir_lowering=False, debug=False)

    widths = [pair_layout(rank_lens, j)[3] for j in range(pairs)]
    qk_w = [pair_layout(rank_lens, j)[2][0] for j in range(pairs)]
    pk = [
        nc.dram_tensor(f"pk{j}", [128, widths[j]], MM_DT, kind="ExternalInput").ap()
        for j in range(pairs)
    ]
    out = nc.dram_tensor("out", [slots * 2 * 128, 4 * (D + 1)], F32, kind="ExternalOutput").ap()

    Exp = mybir.ActivationFunctionType.Exp
    exp_eng, epi_eng = assign_engines(rank_lens, quotas)

    with tile.TileContext(nc) as tc, ExitStack() as ctx:
        inp = ctx.enter_context(tc.tile_pool(name="inp", bufs=1))
        ppool = ctx.enter_context(tc.tile_pool(name="pp", bufs=cf["pp"]))
        zpool = ctx.enter_context(tc.tile_pool(name="zp", bufs=cf["zp"]))
        hpool = ctx.enter_context(tc.tile_pool(name="hp", bufs=cf["hp"]))
        epool = ctx.enter_context(tc.tile_pool(name="ep", bufs=cf["ep"]))
        spool = ctx.enter_context(tc.tile_pool(name="sp", bufs=cf["sp"], space="PSUM"))
        apool = ctx.enter_context(tc.tile_pool(name="ap", bufs=cf["ap"], space="PSUM"))

        order = cf.get("order") or slot_order(rank_lens)
        # DMA pairs in execution order so the first-executed slot's data
        # lands first
        pair_order = []
        for s in order:
            if s // 2 not in pair_order:
                pair_order.append(s // 2)

        qk_t = [None] * pairs
        km_t = [None] * pairs   # middle k-columns (3-way-split first pair)
        kx_t = [None] * pairs   # overflow k-columns beyond the shorter slot
        kx_at = [None] * pairs  # k-tile index where the overflow tile starts
        v_t = [None] * pairs
        j0 = pair_order[0]
        dma_seq = cf.get("dma_seq") or [
            (kind, j) for j in pair_order for kind in ("qk", "v")
        ]
        for kind, j in dma_seq:
            if kind == "qk":
                na, nb = rank_lens[2 * j], rank_lens[2 * j + 1]
                if j in pair_order[:2] and na > nb and nb > 1:
                    # 3-way split for the startup-critical first pairs: the
                    # very first matmul waits only for q + one k-tile.
                    w1 = QS + KT
                    qk_t[j] = inp.tile([128, w1], MM_DT, name=f"qk{j}")
                    nc.sync.dma_start(qk_t[j][:], pk[j][:, :w1])
                    km_t[j] = inp.tile([128, (nb - 1) * KT], MM_DT, name=f"km{j}")
                    nc.sync.dma_start(km_t[j][:], pk[j][:, w1 : QS + nb * KT])
                    kx_t[j] = inp.tile([128, (na - nb) * KT], MM_DT, name=f"kx{j}")
                    kx_at[j] = nb
                    nc.sync.dma_start(kx_t[j][:], pk[j][:, QS + nb * KT : qk_w[j]])
                elif na > nb and (j == j0 or cf.get("split_all", True)):
                    wa = QS + nb * KT
                    qk_t[j] = inp.tile([128, wa], MM_DT, name=f"qk{j}")
                    nc.sync.dma_start(qk_t[j][:], pk[j][:, :wa])
                    kx_t[j] = inp.tile([128, (na - nb) * KT], MM_DT, name=f"kx{j}")
                    kx_at[j] = nb
                    nc.sync.dma_start(kx_t[j][:], pk[j][:, wa : qk_w[j]])
                else:
                    qk_t[j] = inp.tile([128, qk_w[j]], MM_DT, name=f"qk{j}")
                    nc.sync.dma_start(qk_t[j][:], pk[j][:, : qk_w[j]])
            else:
                v_t[j] = inp.tile([128, widths[j] - qk_w[j]], MM_DT, name=f"v{j}")
                nc.sync.dma_start(v_t[j][:], pk[j][:, qk_w[j] :])

        # PE p-state warmup: the cost model ramps the PE to full clock only
        # after ~3us of busy time, and the ramp persists; burn it on dummy
        # matmuls during the input-DMA window so real matmuls run at speed.
        warm_in = inp.tile([64, 128], MM_DT, name="warm_in")
        nc.vector.memset(warm_in[:], 0.0)
        warm_ps = apool.tile([128, 4 * (D + 1)], F32, name="warm_ps", tag="acc")
        for _ in range(cf.get("warm", 18)):
            nc.tensor.matmul(
                warm_ps[:, 0:128], warm_in[:], warm_in[:], start=True, stop=True
            )

        h_defer = None

        def flush_h(rec):
            j0, p4, tier = rec
            j2 = hpool.tile([128, QS], I16, name="j2", tag="j2")
            nc.vector.tensor_scalar(
                j2[:], j0[:], -512, None,
                mybir.AluOpType.add, mybir.AluOpType.bypass,
            )
            eng = nc.gpsimd if tier == "HP" else nc.vector
            eng.tensor_tensor(
                p4[:], j0[:].bitcast(F16), j2[:].bitcast(F16), mybir.AluOpType.add
            )

        def emit_group(pend, g):
            """One MM2 accumulation group (qq, chunk) of a finished slot:
            contracts over all its k-tiles, then closes.  Contraction order
            is by tier readiness (A first, Pool-assisted H last) so a
            lagging exp never head-of-line-blocks the in-order PE."""
            s_p, j_p, voff_p, accs_p, paps_p, ns_p = pend
            qq, c = g // 4, g % 4
            prio = {"A": 0, "D": 1, "H": 2, "HP": 3}
            kts = sorted(range(ns_p), key=lambda kt: prio[exp_eng[(s_p, kt)]])
            for i, kt in enumerate(kts):
                w = v_t[j_p][
                    :, voff_p + kt * (D + 1) : voff_p + (kt + 1) * (D + 1)
                ]
                nc.tensor.matmul(
                    accs_p[qq][:, c * (D + 1) : (c + 1) * (D + 1)],
                    paps_p[kt][:, g * 128 : (g + 1) * 128],
                    w,
                    start=(i == 0),
                    stop=(i == ns_p - 1),
                )

        def emit_epilogue(pend):
            s_p = pend[0]
            accs_p = pend[3]
            for qq in range(2):
                acc_sb = epool.tile([128, 4 * (D + 1)], F32, name="acc_sb")
                if epi_eng[(s_p, qq)] == "A":
                    nc.scalar.copy(acc_sb[:], accs_p[qq][:])
                else:
                    nc.vector.tensor_copy(acc_sb[:], accs_p[qq][:])
                r0 = (s_p * 2 + qq) * 128
                nc.sync.dma_start(out[r0 : r0 + 128, :], acc_sb[:])

        pending = None   # previous slot's record awaiting its MM2 sweep
        for s in [s for _ in range(repeat) for s in order]:
            ns = rank_lens[s]
            j = s // 2          # pair index (shared input tile)
            pb = (s % 2) * 64   # partition base for q/k sections
            qo, ko, vo, _ = pair_layout(rank_lens, j)
            voff = vo[s % 2] - qk_w[j]
            pt = qk_t[j]
            accs = [
                apool.tile([128, 4 * (D + 1)], F32, name=f"acc{s}_{qq}", tag="acc")
                for qq in range(2)
            ]

            p_aps = []
            done_g = 0
            for kt in range(ns):
                s_ps = spool.tile([128, QS], F32, name="s_ps")
                if kx_at[j] is not None and kt >= kx_at[j]:
                    kk = kt - kx_at[j]
                    lhsT = kx_t[j][pb : pb + 64, kk * KT : (kk + 1) * KT]
                elif km_t[j] is not None and kt >= 1:
                    lhsT = km_t[j][pb : pb + 64, (kt - 1) * KT : kt * KT]
                else:
                    lhsT = pt[pb : pb + 64, ko + kt * KT : ko + (kt + 1) * KT]
                for qq in range(2):
                    nc.tensor.matmul(
                        s_ps[:, qq * 512 : (qq + 1) * 512],
                        lhsT,
                        pt[pb : pb + 64, qo + qq * 512 : qo + (qq + 1) * 512],
                        start=True,
                        stop=True,
                    )
                tier = exp_eng[(s, kt)]
                if tier == "A":
                    p_t = ppool.tile([128, QS], F16, name="p_t")
                    nc.scalar.activation(p_t[:], s_ps[:], Exp, scale=SCALE)
                    p_aps.append(p_t[:])
                elif tier == "D":
                    z_t = zpool.tile([128, QS], I16, name="z_t")
                    nc.vector.tensor_scalar(
                        z_t[:], s_ps[:], C_TRICK, MAGIC_D,
                        mybir.AluOpType.mult, mybir.AluOpType.add,
                    )
                    p_aps.append(z_t[:].bitcast(F16))
                else:  # H / HP: 2-phase averaged trick
                    j0 = hpool.tile([128, QS], I16, name="j0", tag="j0")
                    nc.vector.tensor_scalar(
                        j0[:], s_ps[:], C_TRICK, MAGIC_H,
                        mybir.AluOpType.mult, mybir.AluOpType.add,
                    )
                    p4 = ppool.tile([128, QS], F16, name="p4", tag="p4")
                    flush_h((j0, p4, tier))
                    p_aps.append(p4[:])

                # Splice the previous slot's MM2 sweep groups between this
                # slot's MM1/exp stages so the in-order PE overlaps the
                # sweep with the exp engines instead of serializing.  The
                # one-stage lag keeps the first group (which waits on the
                # previous slot's LAST exp) from stalling the in-order PE
                # ahead of ready MM1s.
                if pending is not None:
                    want = (8 * max(0, kt - 3)) // max(1, ns - 3)
                    while done_g < min(want, 8):
                        emit_group(pending, done_g)
                        done_g += 1

            if h_defer is not None:  # sweep of this slot (emitted next
                flush_h(h_defer)     # slot) must see op4 already emitted
                h_defer = None
            if pending is not None:
                while done_g < 8:
                    emit_group(pending, done_g)
                    done_g += 1
                emit_epilogue(pending)
            pending = (s, j, voff, accs, p_aps, ns)

        # drain the final slot's sweep + epilogue
        for g in range(8):
            emit_group(pending, g)
        emit_epilogue(pending)

    nc.compile()
    return nc


def plan_and_pack(queries, keys, values, valid_lens):
    """Split jobs into k-segments, deal into rank slots, gather fp16 inputs."""
    q = np.ascontiguousarray(np.asarray(queries, dtype=np.float32))
    k = np.asarray(keys, dtype=np.float32)
    v = np.asarray(values, dtype=np.float32)
    vl = np.asarray(valid_lens, dtype=np.int64)

    nkt = np.maximum(1, -(-vl // KT))  # ceil

    def make_segs(seg_max):
        segs = []  # (len, b, qh, k0)
        for b in range(B):
            n = int(nkt[b])
            m = -(-n // seg_max)
            base, rem = divmod(n, m)
            sizes = [base + 1] * rem + [base] * (m - rem)
            for qh in range(LQ // QS):
                k0 = 0
                for sz in sizes:
                    segs.append((sz, b, qh, k0))
                    k0 += sz
        segs.sort(key=lambda t: (-t[0], t[1], t[2], t[3]))
        return segs

    def cost(segs):
        ls = sorted((s[0] for s in segs), reverse=True)
        while len(ls) % N_CORES:
            ls.append(0)
        slots = len(ls) // N_CORES
        if slots % 2:
            slots += 1
            ls += [0] * N_CORES
        rsum = sum(max(ls[N_CORES * r], 1) for r in range(slots))
        return rsum * 0.68 + slots * 0.55  # us: PE-paced unit + slot overhead

    seg_best = min(range(5, SEG + 1), key=lambda m: cost(make_segs(m)))
    segs = make_segs(seg_best)
    while len(segs) % N_CORES:
        segs.append(None)
    slots = len(segs) // N_CORES
    if slots % 2:  # pair structure needs an even slot count
        segs.extend([None] * N_CORES)
        slots += 1
    rank_lens = []
    quotas = []
    for r in range(slots):
        rsegs = segs[N_CORES * r : N_CORES * (r + 1)]
        first = rsegs[0]
        ns = first[0] if first is not None else 1
        rank_lens.append(ns)
        min_vl = min(
            (int(vl[sg[1]]) for sg in rsegs if sg is not None), default=LK
        )
        qd = int(ns * min(1.0, CAP_D * min_vl))
        qh = int(ns * min(1.0, CAP_H * min_vl))
        quotas.append((qd, qh))
    pairs = slots // 2
    exp_eng, _ = assign_engines(rank_lens, quotas)
    vtier = {"A": 1.0, "D": V_SCALE_D, "H": V_SCALE_H, "HP": V_SCALE_H}

    kT = np.swapaxes(k, 1, 2)  # [B, D, LK] view
    parts = np.arange(KT)

    in_maps = []
    slot_map = []  # per core: [(b, qh, k0) or None, ...] per slot
    for c in range(N_CORES):
        core_map = {}
        smap = []
        for j in range(pairs):
            qo, ko, vo, width = pair_layout(rank_lens, j)
            pkj = np.zeros((128, width), dtype=np.float16)
            for i, s in enumerate((2 * j, 2 * j + 1)):
                nr = rank_lens[s]  # compiled (padded) slot length
                seg = segs[N_CORES * s + c]
                if seg is None:
                    smap.append(None)
                    continue
                sz, b, qh, k0 = seg
                pb = i * 64
                smap.append((b, qh, k0))
                pkj[pb : pb + 64, qo : qo + QS] = q[b, qh * QS : (qh + 1) * QS, :].T
                kw = min(nr * KT, LK - k0 * KT)
                pkj[pb : pb + 64, ko : ko + kw] = kT[b, :, k0 * KT : k0 * KT + kw]
                vslab = pkj[:, vo[i] : vo[i] + nr * (D + 1)].reshape(128, nr, D + 1)
                nv = kw // KT
                vslab[:, :nv, :D] = (
                    v[b, k0 * KT : k0 * KT + nv * KT, :]
                    .reshape(nv, KT, D)
                    .transpose(1, 0, 2)
                )
                vslab[:, :, D] = 1.0
                # fold each tier's constant P factor into V (exact)
                for kt_i in range(nr):
                    sc = vtier[exp_eng[(s, kt_i)]]
                    if sc != 1.0:
                        vslab[:, kt_i, :] = vslab[:, kt_i, :].astype(np.float32) * sc
                # zero contributions of masked keys and keys outside the
                # segment's own range [k0, k0+sz)
                kid = (k0 + np.arange(nr))[None, :] * KT + parts[:, None]
                dead = (kid >= vl[b]) | (kid >= (k0 + sz) * KT)
                vslab[dead] = 0.0
            core_map[f"pk{j}"] = pkj
        in_maps.append(core_map)
        slot_map.append(smap)
    return (rank_lens, quotas), in_maps, slot_map


def scatter_out(results, slot_map):
    acc = {}  # (b, qh) -> [QS, 65] float64 partial sums
    for c in range(N_CORES):
        oc = results[c]["out"]
        for s, seg in enumerate(slot_map[c]):
            if seg is None:
                continue
            b, qh, _ = seg
            blk = oc[s * 256 : (s + 1) * 256, :].astype(np.float64)
            # rows: [2 qq x 128 p], cols: [4 chunks x 65] -> [1024 q, 65]
            jb = (
                blk.reshape(2, 128, 4, D + 1)
                .transpose(0, 2, 1, 3)
                .reshape(QS, D + 1)
            )
            key = (b, qh)
            if key in acc:
                acc[key] += jb
            else:
                acc[key] = jb
    out = np.empty((B, LQ, D), dtype=np.float32)
    for (b, qh), a in acc.items():
        out[b, qh * QS : (qh + 1) * QS, :] = a[:, :D] / a[:, D : D + 1]
    return out


def kernel(queries, keys, values, valid_lens, _run=None):
    plan, in_maps, slot_map = plan_and_pack(queries, keys, values, valid_lens)
    nc = build_bass(plan)
    if _run is not None:  # test hook (e.g. CoreSim)
        results = _run(nc, in_maps)
    else:
        import time as _time

        last = None
        for attempt in range(4):  # axon devices flake transiently under load
            try:
                results = bass_utils.run_bass_kernel_spmd(
                    nc, in_maps, core_ids=list(range(N_CORES))
                ).results
                break
            except Exception as e:  # noqa: BLE001
                last = e
                _time.sleep(45.0 * (attempt + 1))
        else:
            raise last
    return scatter_out(results, slot_map)


# revision 53
# speedup vs baseline: 1.2269x; 1.0276x over previous
"""Masked dot-product attention (B=16, LQ=LK=2048, D=64) on 8 TRN2 NeuronCores.

Strategy
--------
out[b] = softmax(mask(Q K^T / 8)) V, keys >= valid_len[b] masked.

Work decomposition: each (batch, q-half-of-1024) job costs
ceil(valid_len/128) k-tiles; jobs split into segments of <= 8 k-tiles,
sorted and dealt 8-at-a-time into slot ranks so all cores run one
instruction stream while total work tracks the sparsity.

Per (slot, k-tile) on device, all inputs fp16:
  MM1   S^T[kk, q] = (K^T chunk).T @ Q^T      (d=64 contraction, f32 acc)
  EXP   P[kk, q]   = exp(S^T / 8) as fp16     (three engine tiers, below)
  MM2   acc[q, dd|1] += P_chunk.T @ [V|1]     (k contraction, PSUM accum)

MM2 uses P 128-column chunks as the *stationary* tensor and [V|ones]
[128,65] as the *moving* tensor: 8x65 moving rows per k-tile instead of
2x512 (matmul time ~ moving rows; fp16 runs 1 PE cycle/row at any size),
cutting MM2 from 854ns to ~220ns/k-tile.  acc comes out q-major.  Since
only one PSUM accumulation group may be open per 2KB bank, each (qq,
chunk) group contracts over all its k-tiles and closes before the next
opens; the whole 8-group sweep is deferred one slot and spliced between
the NEXT slot's MM1/exp stages (in-order PE overlaps it with the exp
engines), with contraction in tier-readiness order so a lagging exp never
head-of-line-blocks the PE.

EXP runs on three engine tiers to beat the ScalarE throughput wall
(ScalarE exp costs 1 elem/cycle/lane; ~36us for this shape on one engine):
 - A (ACT): true exp activation, exact to fp16.
 - D (DVE, 1 op): Schraudolph bit-trick: j = round_i16(s*C + 15360+sig);
   j IS the bit pattern of fp16(~exp(s/8)) with ~1.8% rms error; MM2 reads
   the int16 tile bitcast to fp16.
 - H (DVE + Pool, 3 ops): j0 as above (-1024 bias), j2 = j0-512, then
   P = f16(j0)+f16(j2) (tensor_tensor, on Pool for some tiles): averages
   the interp error at phases half a period apart -> ~0.53% rms.
Each tier's constant bias factor is folded into that k-tile's V slab
host-side (exact), so tiers mix freely.  Per-batch tile-share caps
(prop. to valid_len) keep the end-to-end absmax error ~6e-3 (gate 2e-2);
a greedy balancer assigns tiers under these caps to equalize engine
loads, interleaving tiers within each slot (the s_ps PSUM ring forces
strict k-tile order, so same-engine runs would serialize).

Also: PE p-state warmup matmuls during the input-DMA window (the cost
model's clock ramp persists), fp16 inputs halve DMA, input pair tensors
stream in execution order with the startup-critical first transfers split
fine.  Masking: V rows and the ones-column are zeroed host-side for keys
>= valid_len or outside the segment.  Each slot DMAs its raw [2x128,
4x65] f32 accumulator out; the host sums each job's segments in f64 and
divides (exact).  TimelineSim: ~41.0us vs 47.1us for the previous
ACT-bound f32r kernel.
"""

import math
from contextlib import ExitStack

import numpy as np

import concourse.bacc as bacc
import concourse.mybir as mybir
import concourse.tile as tile
import concourse.bass_utils as bass_utils

B, LQ, LK, D = 16, 2048, 2048, 64
N_CORES = 8
KT = 128          # keys per k-tile
QS = 1024         # queries per slot (q-half)
SEG = 8           # max k-tiles per segment
SCALE = 1.0 / math.sqrt(D)

F32 = mybir.dt.float32
F16 = mybir.dt.float16
I16 = mybir.dt.int16
MM_DT = F16

# Schraudolph fp16 trick constants (input = RAW scores, scale folded in).
# D tier (1 DVE op):  P-bits j = round_i16(s*C_TRICK + 15360 + SIG_D).
# H tier (2-phase sum, 3 ops): j0 = round_i16(s*C_TRICK + 15360-1024+SIG_H);
#   j2 = j0-512; P = f16(j0) + f16(j2) -- a 0.59/0.41-weighted average of
#   the Schraudolph interp error at phases t and t-1/2, cutting the
#   centered rms from ~1.8% to ~0.53%.
# Each tier's P carries a constant total factor E[P/e^x] (measured
# numerically over x~N(0,1)); the host folds 1/factor into that k-tile's V
# slab (V_SCALE_*), so tiers mix exactly.
C_TRICK = SCALE * math.log2(math.e) * 1024.0          # 184.665
SIG_D = -44.0
SIG_H = -44.0
MAGIC_D = 15360.0 + SIG_D
MAGIC_H = 15360.0 - 1024.0 + SIG_H
V_SCALE_D = 1.0 / 1.0101518
V_SCALE_H = 1.0 / 0.8622136
# Trick-exp noise multiplies softmax weights; end-to-end absmax-rel error
# fits err_b ~= coef * sqrt(share_b * 964 / vl_b) with coef 2.1e-2 for the
# D tier (delta_rms 1.77%) and 6.35e-3 for the H tier (0.535%).  Per-batch
# share caps hold each tier's contribution to T1=8e-3 / T2=7.5e-3
# (quadrature sum ~1.1e-2 < 2e-2 gate).
CAP_D = (9.0e-3 / 2.1e-2) ** 2 / 964.0
CAP_H = (1.1e-2 / 6.35e-3) ** 2 / 964.0

# engine cost model (ns) for the greedy balancer (incl. seq overheads,
# calibrated against TimelineSim busy totals)
ACT_EXP_NS = 1100.0
DVE_EXP_NS = 1290.0     # D tier: 1 tensor_scalar, f32 psum -> i16
DVE_H_NS = 1620.0       # H tier DVE ops: op1 + op2
DVE_TT_NS = 650.0       # H op4 on DVE (f16 tensor_tensor, 2x)
POOL_TT_NS = 1650.0     # H op4 on Pool
ACT_CPY_NS = 440.0
DVE_CPY_NS = 450.0


def pair_layout(rank_lens, j):
    """fp16-element column offsets of the sections inside pair tensor j.

    Sections: qT (both slots stacked on partition halves) | kT (same) |
    vp[slot 2j] | vp[slot 2j+1] ([V|1] per k-tile, 65 cols each, zeroed for
    masked / out-of-segment keys).
    """
    na, nb = rank_lens[2 * j], rank_lens[2 * j + 1]
    qo = 0
    ko = qo + QS
    vo = [ko + na * KT, ko + na * KT + na * (D + 1)]
    width = vo[1] + nb * (D + 1)
    return qo, ko, vo, width


def slot_order(rank_lens):
    """Execution order: shortest slots first (less DMA before compute)."""
    return sorted(range(len(rank_lens)), key=lambda s: rank_lens[s])


def assign_engines(rank_lens, quotas):
    """Three-tier exp schedule under per-slot accuracy quotas.

    quotas[s] = (qd, qh): max D-tier / H-tier tiles in slot s.  Greedy
    minimizes the max engine load; 'H' tiles split their op4 between Pool
    and DVE.  Returns (exp_eng {(s,kt): 'A'|'D'|'H'|'HP'}, epi_eng
    {(s,qq): 'A'|'D'}).  'H' = op4 on DVE, 'HP' = op4 on Pool.
    """
    exp_eng = {}
    epi_eng = {}
    load = {"A": 0.0, "D": 0.0, "P": 0.0}
    order = slot_order(rank_lens)
    for si, s in enumerate(order):
        ns = rank_lens[s]
        qd, qh = quotas[s]
        # The final slot's DVE/Pool units drain serially after ACT is done
        # (multi-op chains have long latency); bias its mix toward ACT.
        ob = 4000.0 if si == len(order) - 1 else 0.0
        # pass 1: per-slot tier counts by global engine-load balance
        counts = {"A": 0, "D": 0, "H": 0, "HP": 0}
        for _ in range(ns):
            cands = [("A", load["A"] + ACT_EXP_NS, ACT_EXP_NS, 0.0)]
            if counts["D"] < qd:
                cands.append(("D", load["D"] + ob + DVE_EXP_NS, DVE_EXP_NS, 0.0))
            if counts["H"] + counts["HP"] < qh:
                if load["P"] + POOL_TT_NS <= load["D"] + DVE_H_NS + 1000.0:
                    cands.append(
                        (
                            "HP",
                            ob + max(load["D"] + DVE_H_NS, load["P"] + POOL_TT_NS),
                            DVE_H_NS,
                            POOL_TT_NS,
                        )
                    )
                cands.append(
                    (
                        "H",
                        load["D"] + ob + DVE_H_NS + DVE_TT_NS,
                        DVE_H_NS + DVE_TT_NS,
                        0.0,
                    )
                )
            t, _, dcost, pcost = min(cands, key=lambda c: c[1])
            counts[t] += 1
            if t == "A":
                load["A"] += ACT_EXP_NS
            else:
                load["D"] += dcost
                load["P"] += pcost
        # pass 2: interleave tiers evenly within the slot (strict kt-order
        # processing through the s_ps ring serializes same-engine runs)
        seq = []
        rem = dict(counts)
        placed = {t: 0 for t in rem}
        for kt in range(ns):
            t = max(
                (t for t in rem if rem[t] > 0),
                key=lambda t: rem[t] / (placed[t] + 1),
            )
            seq.append(t)
            rem[t] -= 1
            placed[t] += 1
        for kt, t in enumerate(seq):
            exp_eng[(s, kt)] = t
        for qq in range(2):
            if load["A"] + ACT_CPY_NS <= load["D"] + DVE_CPY_NS:
                epi_eng[(s, qq)] = "A"
                load["A"] += ACT_CPY_NS
            else:
                epi_eng[(s, qq)] = "D"
                load["D"] += DVE_CPY_NS
    return exp_eng, epi_eng


def exp_model(tier, sraw):
    """Numpy model of the device exp for one tile: sraw [128, nq] raw
    scores (f32), returns P as float64 (matching fp16/bit-trick device
    numerics).  Used by the packer (V prescale is exact regardless) and by
    test emulation."""
    if tier == "A":
        return (
            np.exp(sraw.astype(np.float32) * np.float32(SCALE))
            .astype(np.float16)
            .astype(np.float64)
        )
    prod = (sraw.astype(np.float32) * np.float32(C_TRICK)).astype(np.float32)
    if tier == "D":
        j = np.round(prod.astype(np.float64) + MAGIC_D).astype(np.int64)
        return (
            np.ascontiguousarray(j.astype(np.int16)).view(np.float16).astype(np.float64)
        )
    j0 = np.round(prod.astype(np.float64) + MAGIC_H).astype(np.int64)
    t1 = np.ascontiguousarray(j0.astype(np.int16)).view(np.float16)
    t2 = np.ascontiguousarray((j0 - 512).astype(np.int16)).view(np.float16)
    return (t1 + t2).astype(np.float64)


def build_bass(plan, repeat=1, cfg=None):
    """Build the per-core Bass program.

    plan = (rank_lens, quotas): compiled slot lengths + per-slot DVE
    trick-exp tile quotas.  repeat>1 re-runs everything (timing only).
    """
    rank_lens, quotas = plan
    cf = {"sp": 3, "ap": 2, "pp": 24, "zp": 24, "hp": 6, "ep": 6}
    if cfg:
        cf.update(cfg)
    slots = len(rank_lens)
    pairs = slots // 2
    nc = bacc.Bacc("TRN2", target_bir_lowering=False, debug=False)

    widths = [pair_layout(rank_lens, j)[3] for j in range(pairs)]
    qk_w = [pair_layout(rank_lens, j)[2][0] for j in range(pairs)]
    pk = [
        nc.dram_tensor(f"pk{j}", [128, widths[j]], MM_DT, kind="ExternalInput").ap()
        for j in range(pairs)
    ]
    out = nc.dram_tensor("out", [slots * 2 * 128, 4 * (D + 1)], F32, kind="ExternalOutput").ap()

    Exp = mybir.ActivationFunctionType.Exp
    exp_eng, epi_eng = assign_engines(rank_lens, quotas)

    with tile.TileContext(nc) as tc, ExitStack() as ctx:
        inp = ctx.enter_context(tc.tile_pool(name="inp", bufs=1))
        ppool = ctx.enter_context(tc.tile_pool(name="pp", bufs=cf["pp"]))
        zpool = ctx.enter_context(tc.tile_pool(name="zp", bufs=cf["zp"]))
        hpool = ctx.enter_context(tc.tile_pool(name="hp", bufs=cf["hp"]))
        epool = ctx.enter_context(tc.tile_pool(name="ep", bufs=cf["ep"]))
        spool = ctx.enter_context(tc.tile_pool(name="sp", bufs=cf["sp"], space="PSUM"))
        apool = ctx.enter_context(tc.tile_pool(name="ap", bufs=cf["ap"], space="PSUM"))

        order = cf.get("order") or slot_order(rank_lens)
        # DMA pairs in execution order so the first-executed slot's data
        # lands first
        pair_order = []
        for s in order:
            if s // 2 not in pair_order:
                pair_order.append(s // 2)

        qk_t = [None] * pairs
        q0_t = [None] * pairs   # finest-split first pair: q half 0
        q1_t = [None] * pairs   # ... q half 1
        k1_t = [None] * pairs   # ... k-tile 0
        km_t = [None] * pairs   # middle k-columns (3-way-split first pair)
        kx_t = [None] * pairs   # overflow k-columns beyond the shorter slot
        kx_at = [None] * pairs  # k-tile index where the overflow tile starts
        v_t = [None] * pairs
        j0 = pair_order[0]
        dma_seq = cf.get("dma_seq") or [
            (kind, j) for j in pair_order for kind in ("qk", "v")
        ]
        for kind, j in dma_seq:
            if kind == "qk":
                na, nb = rank_lens[2 * j], rank_lens[2 * j + 1]
                if j in pair_order[:2] and na > nb and nb > 1:
                    # 3-way split for the startup-critical first pairs: the
                    # very first matmul waits only for q + one k-tile.
                    w1 = QS + KT
                    qk_t[j] = inp.tile([128, w1], MM_DT, name=f"qk{j}")
                    nc.sync.dma_start(qk_t[j][:], pk[j][:, :w1])
                    km_t[j] = inp.tile([128, (nb - 1) * KT], MM_DT, name=f"km{j}")
                    nc.sync.dma_start(km_t[j][:], pk[j][:, w1 : QS + nb * KT])
                    kx_t[j] = inp.tile([128, (na - nb) * KT], MM_DT, name=f"kx{j}")
                    kx_at[j] = nb
                    nc.sync.dma_start(kx_t[j][:], pk[j][:, QS + nb * KT : qk_w[j]])
                elif na > nb and (j == j0 or cf.get("split_all", True)):
                    wa = QS + nb * KT
                    qk_t[j] = inp.tile([128, wa], MM_DT, name=f"qk{j}")
                    nc.sync.dma_start(qk_t[j][:], pk[j][:, :wa])
                    kx_t[j] = inp.tile([128, (na - nb) * KT], MM_DT, name=f"kx{j}")
                    kx_at[j] = nb
                    nc.sync.dma_start(kx_t[j][:], pk[j][:, wa : qk_w[j]])
                else:
                    qk_t[j] = inp.tile([128, qk_w[j]], MM_DT, name=f"qk{j}")
                    nc.sync.dma_start(qk_t[j][:], pk[j][:, : qk_w[j]])
            else:
                v_t[j] = inp.tile([128, widths[j] - qk_w[j]], MM_DT, name=f"v{j}")
                nc.sync.dma_start(v_t[j][:], pk[j][:, qk_w[j] :])

        # PE p-state warmup: the cost model ramps the PE to full clock only
        # after ~3us of busy time, and the ramp persists; burn it on dummy
        # matmuls during the input-DMA window so real matmuls run at speed.
        warm_in = inp.tile([64, 128], MM_DT, name="warm_in")
        nc.vector.memset(warm_in[:], 0.0)
        warm_ps = apool.tile([128, 4 * (D + 1)], F32, name="warm_ps", tag="acc")
        for _ in range(cf.get("warm", 18)):
            nc.tensor.matmul(
                warm_ps[:, 0:128], warm_in[:], warm_in[:], start=True, stop=True
            )

        h_defer = None

        def flush_h(rec):
            j0, p4, tier = rec
            j2 = hpool.tile([128, QS], I16, name="j2", tag="j2")
            nc.vector.tensor_scalar(
                j2[:], j0[:], -512, None,
                mybir.AluOpType.add, mybir.AluOpType.bypass,
            )
            eng = nc.gpsimd if tier == "HP" else nc.vector
            eng.tensor_tensor(
                p4[:], j0[:].bitcast(F16), j2[:].bitcast(F16), mybir.AluOpType.add
            )

        def emit_group(pend, g):
            """One MM2 accumulation group (qq, chunk) of a finished slot:
            contracts over all its k-tiles, then closes.  Contraction order
            is by tier readiness (A first, Pool-assisted H last) so a
            lagging exp never head-of-line-blocks the in-order PE."""
            s_p, j_p, voff_p, accs_p, paps_p, ns_p = pend
            qq, c = g // 4, g % 4
            prio = {"A": 0, "D": 1, "H": 2, "HP": 3}
            kts = sorted(range(ns_p), key=lambda kt: prio[exp_eng[(s_p, kt)]])
            for i, kt in enumerate(kts):
                w = v_t[j_p][
                    :, voff_p + kt * (D + 1) : voff_p + (kt + 1) * (D + 1)
                ]
                nc.tensor.matmul(
                    accs_p[qq][:, c * (D + 1) : (c + 1) * (D + 1)],
                    paps_p[kt][:, g * 128 : (g + 1) * 128],
                    w,
                    start=(i == 0),
                    stop=(i == ns_p - 1),
                )

        def emit_epilogue(pend):
            s_p = pend[0]
            accs_p = pend[3]
            for qq in range(2):
                acc_sb = epool.tile([128, 4 * (D + 1)], F32, name="acc_sb")
                if epi_eng[(s_p, qq)] == "A":
                    nc.scalar.copy(acc_sb[:], accs_p[qq][:])
                else:
                    nc.vector.tensor_copy(acc_sb[:], accs_p[qq][:])
                r0 = (s_p * 2 + qq) * 128
                nc.sync.dma_start(out[r0 : r0 + 128, :], acc_sb[:])

        pending = None   # previous slot's record awaiting its MM2 sweep
        for s in [s for _ in range(repeat) for s in order]:
            ns = rank_lens[s]
            j = s // 2          # pair index (shared input tile)
            pb = (s % 2) * 64   # partition base for q/k sections
            qo, ko, vo, _ = pair_layout(rank_lens, j)
            voff = vo[s % 2] - qk_w[j]
            pt = qk_t[j]
            accs = [
                apool.tile([128, 4 * (D + 1)], F32, name=f"acc{s}_{qq}", tag="acc")
                for qq in range(2)
            ]

            p_aps = []
            done_g = 0
            for kt in range(ns):
                s_ps = spool.tile([128, QS], F32, name="s_ps")
                if kx_at[j] is not None and kt >= kx_at[j]:
                    kk = kt - kx_at[j]
                    lhsT = kx_t[j][pb : pb + 64, kk * KT : (kk + 1) * KT]
                elif k1_t[j] is not None and kt == 0:
                    lhsT = k1_t[j][pb : pb + 64, :]
                elif km_t[j] is not None and kt >= 1:
                    lhsT = km_t[j][pb : pb + 64, (kt - 1) * KT : kt * KT]
                else:
                    lhsT = pt[pb : pb + 64, ko + kt * KT : ko + (kt + 1) * KT]
                for qq in range(2):
                    if q0_t[j] is not None:
                        rhs = (q0_t[j] if qq == 0 else q1_t[j])[
                            pb : pb + 64, 0:512
                        ]
                    else:
                        rhs = pt[pb : pb + 64, qo + qq * 512 : qo + (qq + 1) * 512]
                    nc.tensor.matmul(
                        s_ps[:, qq * 512 : (qq + 1) * 512],
                        lhsT,
                        rhs,
                        start=True,
                        stop=True,
                    )
                tier = exp_eng[(s, kt)]
                if tier == "A":
                    p_t = ppool.tile([128, QS], F16, name="p_t")
                    nc.scalar.activation(p_t[:], s_ps[:], Exp, scale=SCALE)
                    p_aps.append(p_t[:])
                elif tier == "D":
                    z_t = zpool.tile([128, QS], I16, name="z_t")
                    nc.vector.tensor_scalar(
                        z_t[:], s_ps[:], C_TRICK, MAGIC_D,
                        mybir.AluOpType.mult, mybir.AluOpType.add,
                    )
                    p_aps.append(z_t[:].bitcast(F16))
                else:  # H / HP: 2-phase averaged trick
                    j0 = hpool.tile([128, QS], I16, name="j0", tag="j0")
                    nc.vector.tensor_scalar(
                        j0[:], s_ps[:], C_TRICK, MAGIC_H,
                        mybir.AluOpType.mult, mybir.AluOpType.add,
                    )
                    p4 = ppool.tile([128, QS], F16, name="p4", tag="p4")
                    flush_h((j0, p4, tier))
                    p_aps.append(p4[:])

                # Splice the previous slot's MM2 sweep groups between this
                # slot's MM1/exp stages so the in-order PE overlaps the
                # sweep with the exp engines instead of serializing.  The
                # one-stage lag keeps the first group (which waits on the
                # previous slot's LAST exp) from stalling the in-order PE
                # ahead of ready MM1s.
                if pending is not None:
                    want = (8 * max(0, kt - 3)) // max(1, ns - 3)
                    while done_g < min(want, 8):
                        emit_group(pending, done_g)
                        done_g += 1

            if h_defer is not None:  # sweep of this slot (emitted next
                flush_h(h_defer)     # slot) must see op4 already emitted
                h_defer = None
            if pending is not None:
                while done_g < 8:
                    emit_group(pending, done_g)
                    done_g += 1
                emit_epilogue(pending)
            pending = (s, j, voff, accs, p_aps, ns)

        # drain the final slot's sweep + epilogue
        for g in range(8):
            emit_group(pending, g)
        emit_epilogue(pending)

    nc.compile()
    return nc


def plan_and_pack(queries, keys, values, valid_lens):
    """Split jobs into k-segments, deal into rank slots, gather fp16 inputs."""
    q = np.ascontiguousarray(np.asarray(queries, dtype=np.float32))
    k = np.asarray(keys, dtype=np.float32)
    v = np.asarray(values, dtype=np.float32)
    vl = np.asarray(valid_lens, dtype=np.int64)

    nkt = np.maximum(1, -(-vl // KT))  # ceil

    def make_segs(seg_max):
        segs = []  # (len, b, qh, k0)
        for b in range(B):
            n = int(nkt[b])
            m = -(-n // seg_max)
            base, rem = divmod(n, m)
            sizes = [base + 1] * rem + [base] * (m - rem)
            for qh in range(LQ // QS):
                k0 = 0
                for sz in sizes:
                    segs.append((sz, b, qh, k0))
                    k0 += sz
        segs.sort(key=lambda t: (-t[0], t[1], t[2], t[3]))
        return segs

    def cost(segs):
        ls = sorted((s[0] for s in segs), reverse=True)
        while len(ls) % N_CORES:
            ls.append(0)
        slots = len(ls) // N_CORES
        if slots % 2:
            slots += 1
            ls += [0] * N_CORES
        rsum = sum(max(ls[N_CORES * r], 1) for r in range(slots))
        return rsum * 0.68 + slots * 0.55  # us: PE-paced unit + slot overhead

    seg_best = min(range(5, SEG + 1), key=lambda m: cost(make_segs(m)))
    segs = make_segs(seg_best)
    while len(segs) % N_CORES:
        segs.append(None)
    slots = len(segs) // N_CORES
    if slots % 2:  # pair structure needs an even slot count
        segs.extend([None] * N_CORES)
        slots += 1
    rank_lens = []
    quotas = []
    for r in range(slots):
        rsegs = segs[N_CORES * r : N_CORES * (r + 1)]
        first = rsegs[0]
        ns = first[0] if first is not None else 1
        rank_lens.append(ns)
        min_vl = min(
            (int(vl[sg[1]]) for sg in rsegs if sg is not None), default=LK
        )
        qd = int(ns * min(1.0, CAP_D * min_vl))
        qh = int(ns * min(1.0, CAP_H * min_vl))
        quotas.append((qd, qh))
    pairs = slots // 2
    exp_eng, _ = assign_engines(rank_lens, quotas)
    vtier = {"A": 1.0, "D": V_SCALE_D, "H": V_SCALE_H, "HP": V_SCALE_H}

    kT = np.swapaxes(k, 1, 2)  # [B, D, LK] view
    parts = np.arange(KT)

    in_maps = []
    slot_map = []  # per core: [(b, qh, k0) or None, ...] per slot
    for c in range(N_CORES):
        core_map = {}
        smap = []
        for j in range(pairs):
            qo, ko, vo, width = pair_layout(rank_lens, j)
            pkj = np.zeros((128, width), dtype=np.float16)
            for i, s in enumerate((2 * j, 2 * j + 1)):
                nr = rank_lens[s]  # compiled (padded) slot length
                seg = segs[N_CORES * s + c]
                if seg is None:
                    smap.append(None)
                    continue
                sz, b, qh, k0 = seg
                pb = i * 64
                smap.append((b, qh, k0))
                pkj[pb : pb + 64, qo : qo + QS] = q[b, qh * QS : (qh + 1) * QS, :].T
                kw = min(nr * KT, LK - k0 * KT)
                pkj[pb : pb + 64, ko : ko + kw] = kT[b, :, k0 * KT : k0 * KT + kw]
                vslab = pkj[:, vo[i] : vo[i] + nr * (D + 1)].reshape(128, nr, D + 1)
                nv = kw // KT
                vslab[:, :nv, :D] = (
                    v[b, k0 * KT : k0 * KT + nv * KT, :]
                    .reshape(nv, KT, D)
                    .transpose(1, 0, 2)
                )
                vslab[:, :, D] = 1.0
                # fold each tier's constant P factor into V (exact)
                for kt_i in range(nr):
                    sc = vtier[exp_eng[(s, kt_i)]]
                    if sc != 1.0:
                        vslab[:, kt_i, :] = vslab[:, kt_i, :].astype(np.float32) * sc
                # zero contributions of masked keys and keys outside the
                # segment's own range [k0, k0+sz)
                kid = (k0 + np.arange(nr))[None, :] * KT + parts[:, None]
                dead = (kid >= vl[b]) | (kid >= (k0 + sz) * KT)
                vslab[dead] = 0.0
            core_map[f"pk{j}"] = pkj
        in_maps.append(core_map)
        slot_map.append(smap)
    return (rank_lens, quotas), in_maps, slot_map


def scatter_out(results, slot_map):
    acc = {}  # (b, qh) -> [QS, 65] float64 partial sums
    for c in range(N_CORES):
        oc = results[c]["out"]
        for s, seg in enumerate(slot_map[c]):
            if seg is None:
                continue
            b, qh, _ = seg
            blk = oc[s * 256 : (s + 1) * 256, :].astype(np.float64)
            # rows: [2 qq x 128 p], cols: [4 chunks x 65] -> [1024 q, 65]
            jb = (
                blk.reshape(2, 128, 4, D + 1)
                .transpose(0, 2, 1, 3)
                .reshape(QS, D + 1)
            )
            key = (b, qh)
            if key in acc:
                acc[key] += jb
            else:
                acc[key] = jb
    out = np.empty((B, LQ, D), dtype=np.float32)
    for (b, qh), a in acc.items():
        out[b, qh * QS : (qh + 1) * QS, :] = a[:, :D] / a[:, D : D + 1]
    return out


def kernel(queries, keys, values, valid_lens, _run=None):
    plan, in_maps, slot_map = plan_and_pack(queries, keys, values, valid_lens)
    nc = build_bass(plan)
    if _run is not None:  # test hook (e.g. CoreSim)
        results = _run(nc, in_maps)
    else:
        import time as _time

        last = None
        for attempt in range(4):  # axon devices flake transiently under load
            try:
                results = bass_utils.run_bass_kernel_spmd(
                    nc, in_maps, core_ids=list(range(N_CORES))
                ).results
                break
            except Exception as e:  # noqa: BLE001
                last = e
                _time.sleep(45.0 * (attempt + 1))
        else:
            raise last
    return scatter_out(results, slot_map)
